# revision 1
# baseline (speedup 1.0000x reference)
"""MoE pointwise conv2d kernel for Trainium2 (8 NeuronCores, SPMD data-parallel).

Problem: out[b,o,h,w] = sum_i (sum_e routing[b,e] * weight[e,o,i]) * x[b,i,h,w]
Shapes:  x [64,384,28,28] f32, routing [64,8] f32, weight [8,384,384] f32.

Strategy (per core, 8 samples each):
  - Routing-combine (agg^T[b][i,o] = sum_e r[b,e] * w[e,o,i]) split across
    VectorE and GpSimdE via fused scalar_tensor_tensor MACs, written directly
    in matmul-lhsT layout (partition = i, free = (ki, o)).
  - Per-sample GEMM out[b] = agg_b @ x_b on TensorE, accumulating over 3
    k-tiles in PSUM (fp32).
  - ScalarE evacuates PSUM -> SBUF; HWDGE DMAs stream x in / out back.
  - Default fp16 wire format (x/weights/out on HBM + agg math) halves DMA
    volume and doubles DVE throughput; end-to-end rel err ~7e-4.
    KERNEL_F32=1 selects the fp32(+float32r matmul) build, rel err ~1.6e-4.
"""
import os
import sys

sys.path.insert(0, "/opt/trn_rl_repo")

import numpy as np
from contextlib import ExitStack

B, C_IN, C_OUT, E, H, W = 64, 384, 384, 8, 28, 28
HW = H * W            # 784
N_CORES = 8
BPC = B // N_CORES    # 8 samples per core
KI = C_IN // 128      # 3 k-tiles
MO = C_OUT // 128     # 3 output-partition tiles
NSPLIT = 2            # 784 -> 2 x 392 (<= 512 psum bank limit)
NCH = HW // NSPLIT    # 392
WCOL = KI * C_OUT     # 1152

USE_F16 = os.environ.get("KERNEL_F32", "0") != "1"

_cache = {}


def _build(use_f16=USE_F16, spl=WCOL, reps=1, serialize_reps=False, pair=True, agg_bufs=2, micro=True, quad=False, dense_rw=False, slack=True, slack2=False):
    import concourse.tile as tile
    import concourse.mybir as mybir
    from concourse import bacc
    from concourse.tile import add_dep_helper

    f32 = mybir.dt.float32
    f32r = mybir.dt.float32r
    f16 = mybir.dt.float16
    mult = mybir.AluOpType.mult
    add = mybir.AluOpType.add

    dio = f16 if use_f16 else f32        # wire dtype for wt/x/out
    dmm = f16 if use_f16 else f32r       # matmul operand dtype

    nc = bacc.Bacc("TRN2", target_bir_lowering=False, debug=False)
    x_d = nc.dram_tensor("x", [BPC, C_IN, HW], dio, kind="ExternalInput")
    rw_d = nc.dram_tensor("rw", [128 if dense_rw else 1, BPC * E], f32,
                          kind="ExternalInput")
    wt_d = nc.dram_tensor("wt", [E, 128, WCOL], dio, kind="ExternalInput")
    out_d = nc.dram_tensor("out", [reps * BPC, C_OUT, HW], dio,
                           kind="ExternalOutput")

    with tile.TileContext(nc) as tc:
        with ExitStack() as ctx:
            wt_pool = ctx.enter_context(tc.tile_pool(name="wt", bufs=E))
            rw_pool = ctx.enter_context(tc.tile_pool(name="rw", bufs=2))
            agg_pool = ctx.enter_context(tc.tile_pool(name="agg", bufs=max(agg_bufs, 4 if quad else (3 if slack2 else 2))))
            x_pool = ctx.enter_context(tc.tile_pool(name="xp", bufs=4 if slack2 else (3 if slack else 2)))
            out_pool = ctx.enter_context(tc.tile_pool(name="op", bufs=10 if slack2 else (8 if slack else 6)))
            ps_pool = ctx.enter_context(tc.tile_pool(name="ps", bufs=8 if slack2 else (6 if slack else 4), space="PSUM"))

            prev_out_dmas, cur_out_dmas = [], []
            pair_tiles = {}

            def _fence(inst):
                if serialize_reps:
                    for d in prev_out_dmas:
                        add_dep_helper(inst.ins, d.ins, reason="serialize reps")
                return inst

            for rep, b in ((r, b) for r in range(reps) for b in range(BPC)):
                if b == 0:
                    prev_out_dmas, cur_out_dmas = cur_out_dmas, []
                    rw_sb = rw_pool.tile([128, BPC * E], f32)
                    _fence(nc.sync.dma_start(
                        rw_sb[:],
                        rw_d[:] if dense_rw
                        else rw_d[:].to_broadcast((128, BPC * E))))
                    wt_sb, wt_dmas = [], []
                    for e in range(E):
                        t = wt_pool.tile([128, WCOL], dio)
                        wt_dmas.append(_fence(nc.sync.dma_start(t[:], wt_d[e])))
                        wt_sb.append(t)
                # ---- routing combine ----
                # DVE does cols [0:spl) with fused scalar_tensor_tensor MACs
                # (2-byte operands keep the 2x_1p DVE mode). GPSIMD cannot run
                # TensorScalarPtr (walrus rejects Pool), and its tensor_tensor
                # 2-op MAC measured ~33us/invocation WORSE on HW (shared-port
                # lock vs DVE packed modes) — keep spl == WCOL (DVE-only).
                # fp16 accumulator keeps every operand 2-byte -> 2x DVE mode
                GSZ = 4 if quad else 2
                if pair and b % GSZ == 0:
                    # emit the MAC chains of samples (b, b+1) interleaved so
                    # DVE hides each chain's op-to-op dependency latency
                    pr = []
                    for bb in range(b, b + GSZ):
                        a_ = agg_pool.tile([128, WCOL], f16 if use_f16 else f32,
                                           tag="aggT")
                        ar_ = agg_pool.tile([128, WCOL], dmm, tag="aggr")
                        pr.append((bb, a_, ar_))
                    for gi in range(GSZ):
                        pair_tiles[b + gi] = pr[gi][1:]
                    for e in range(E):
                        for bb, a_, ar_ in pr:
                            s = rw_sb[:, bb * E + e:bb * E + e + 1]
                            if e == 0:
                                nc.vector.tensor_scalar(
                                    a_[:], wt_sb[0][:], s, None, mult)
                            elif e < E - 1:
                                nc.vector.scalar_tensor_tensor(
                                    a_[:], wt_sb[e][:], s, a_[:], mult, add)
                            elif micro and b == BPC - GSZ:
                                for k3 in range(KI):
                                    cs = slice(k3 * C_OUT, (k3 + 1) * C_OUT)
                                    nc.vector.scalar_tensor_tensor(
                                        ar_[:, cs], wt_sb[e][:, cs], s,
                                        a_[:, cs], mult, add)
                            else:
                                nc.vector.scalar_tensor_tensor(
                                    ar_[:], wt_sb[e][:], s, a_[:], mult, add)
                if pair:
                    aggT, aggT_r = pair_tiles[b]
                    sc = lambda e: rw_sb[:, b * E + e:b * E + e + 1]
                else:
                    aggT = agg_pool.tile([128, WCOL], f16 if use_f16 else f32)
                    aggT_r = agg_pool.tile([128, WCOL], dmm, tag="aggr")
                    sc = lambda e: rw_sb[:, b * E + e:b * E + e + 1]
                if not pair:
                    nc.vector.tensor_scalar(
                        aggT[:, 0:spl], wt_sb[0][:, 0:spl], sc(0), None, mult
                    )
                    for e in range(1, E - 1):
                        nc.vector.scalar_tensor_tensor(
                            aggT[:, 0:spl], wt_sb[e][:, 0:spl], sc(e),
                            aggT[:, 0:spl], mult, add,
                        )
                    nc.vector.scalar_tensor_tensor(
                        aggT_r[:, 0:spl], wt_sb[E - 1][:, 0:spl], sc(E - 1),
                        aggT[:, 0:spl], mult, add,
                    )
                if spl < WCOL:
                    gw = WCOL - spl
                    gtmp = agg_pool.tile([128, gw], f16 if use_f16 else f32,
                                         tag="gtmp")
                    scb = lambda e: sc(e).to_broadcast((128, gw))
                    nc.gpsimd.tensor_tensor(
                        aggT[:, spl:], wt_sb[0][:, spl:], scb(0), mult)
                    for e in range(1, E - 1):
                        nc.gpsimd.tensor_tensor(
                            gtmp[:], wt_sb[e][:, spl:], scb(e), mult)
                        nc.gpsimd.tensor_tensor(
                            aggT[:, spl:], aggT[:, spl:], gtmp[:], add)
                    nc.gpsimd.tensor_tensor(
                        gtmp[:], wt_sb[E - 1][:, spl:], scb(E - 1), mult)
                    nc.gpsimd.tensor_tensor(
                        aggT_r[:, spl:], aggT[:, spl:], gtmp[:], add)

                # ---- load x_b ----
                x_sb = x_pool.tile([128, KI * HW], dmm)
                for ki in range(KI):
                    src = x_d[b, ki * 128:(ki + 1) * 128, :]
                    xi = _fence(nc.sync.dma_start(x_sb[:, ki * HW:(ki + 1) * HW],
                                                  src if use_f16 else src.bitcast(f32r)))
                    if micro and b < 2:
                        for wd in wt_dmas:
                            add_dep_helper(xi.ins, wd.ins,
                                           reason="x after wt (head trim)")

                # ---- per-sample GEMM ----
                for mo in range(MO):
                    for n in range(NSPLIT):
                        ps = ps_pool.tile([128, NCH], f32)
                        for ki in range(KI):
                            lhs = aggT_r[:, ki * C_OUT + mo * 128:
                                         ki * C_OUT + (mo + 1) * 128]
                            rhs = x_sb[:, ki * HW + n * NCH:
                                       ki * HW + (n + 1) * NCH]
                            nc.tensor.matmul(
                                ps[:], lhs, rhs,
                                start=(ki == 0), stop=(ki == KI - 1),
                            )
                        o_sb = out_pool.tile([128, NCH], dio)
                        nc.scalar.copy(o_sb[:], ps[:])
                        cur_out_dmas.append(nc.sync.dma_start(
                            out_d[rep * BPC + b, mo * 128:(mo + 1) * 128,
                                  n * NCH:(n + 1) * NCH],
                            o_sb[:],
                        ))
    nc.compile()
    return nc


def kernel(x: np.ndarray, routing_weights: np.ndarray, weight: np.ndarray,
           _trace: bool = False):
    from concourse.bass_utils import run_bass_kernel_spmd

    x = np.asarray(x, dtype=np.float32)
    routing_weights = np.ascontiguousarray(np.asarray(routing_weights, dtype=np.float32))
    weight = np.asarray(weight, dtype=np.float32)

    if "nc" not in _cache:
        _cache["nc"] = _build()
    nc = _cache["nc"]

    np_io = np.float16 if USE_F16 else np.float32

    # wt[e, p, ki*384 + o] = weight[e, o, ki*128 + p]
    wt = np.ascontiguousarray(
        weight.reshape(E, C_OUT, KI, 128).transpose(0, 3, 2, 1)
        .reshape(E, 128, WCOL).astype(np_io)
    )
    x_r = np.ascontiguousarray(x.reshape(B, C_IN, HW).astype(np_io))

    in_maps = []
    for c in range(N_CORES):
        sl = slice(c * BPC, (c + 1) * BPC)
        in_maps.append({
            "x": x_r[sl],
            "rw": np.ascontiguousarray(routing_weights[sl].reshape(1, BPC * E)),
            "wt": wt,
        })

    res = run_bass_kernel_spmd(nc, in_maps, core_ids=list(range(N_CORES)),
                               trace=_trace)
    out = np.concatenate([res.results[c]["out"] for c in range(N_CORES)], axis=0)
    if _trace:
        _cache["last_result"] = res
    return out.reshape(B, C_OUT, H, W).astype(np.float32)


if __name__ == "__main__":
    rng = np.random.default_rng(0)
    x = rng.standard_normal((B, C_IN, H, W), dtype=np.float32)
    rw = rng.random((B, E), dtype=np.float32)
    w = rng.standard_normal((E, C_OUT, C_IN), dtype=np.float32)
    got = kernel(x, rw, w)
    agg = np.einsum('be,eoi->boi', rw, w)
    want = np.einsum('boi,bihw->bohw', agg, x.reshape(B, C_IN, H, W))
    err = np.abs(got - want).max() / np.abs(want).max()
    print("rel err:", err)



# revision 35
# speedup vs baseline: 1.8124x; 1.8124x over previous
"""MoE pointwise conv2d kernel for Trainium2 (8 NeuronCores, SPMD data-parallel).

Problem: out[b,o,h,w] = sum_i (sum_e routing[b,e] * weight[e,o,i]) * x[b,i,h,w]
Shapes:  x [64,384,28,28] f32, routing [64,8] f32, weight [8,384,384] f32.

Strategy (per core, 8 samples each):
  - Routing-combine split into DVE tensor_scalar mults (4x_2p packed mode,
    0.25 cyc/elem) + tensor_tensor adds (2x_1p, 0.5 cyc/elem); the fused
    scalar_tensor_tensor MAC has NO fast mode (1.0 cyc/elem) - avoid it.
  - PE-assist: for samples in `pe_samples`, the expert sum runs on TensorE
    as identity-lhsT matmuls accumulating in PSUM (keeps PE busy during the
    weight-stream head and balances DVE).
  - Per-sample GEMM out[b] = aggT^T @ x_b on TensorE, n-innermost so
    consecutive matmuls share the stationary lhsT.
  - ScalarE (Act) evacuates PSUM -> SBUF and takes a few offloaded mults.
  - Coalesced DMAs (1 per x sample, 2 out per sample, 8 wt) to cut HWDGE
    fixed overhead; fp16 wire format halves DMA volume.
"""
import os
import sys

sys.path.insert(0, "/opt/trn_rl_repo")

import numpy as np
from contextlib import ExitStack

B, C_IN, C_OUT, E, H, W = 64, 384, 384, 8, 28, 28
HW = H * W            # 784
N_CORES = 8
BPC = B // N_CORES    # 8 samples per core
KI = C_IN // 128      # 3 k-tiles
MO = C_OUT // 128     # 3 output-partition tiles
NSPLIT = 2            # 784 -> 2 x 392 (<= 512 psum bank limit)
NCH = HW // NSPLIT    # 392
WCOL = KI * C_OUT     # 1152 (ki-major, o-minor)

USE_F16 = os.environ.get("KERNEL_F32", "0") != "1"

_cache = {}


def _build(use_f16=USE_F16, reps=1, serialize_reps=False,
           pe_pairs=(0, 3), act_mult_es=(), pool_add_lvls=(), out_split=3,
           combine_order=(0, 3, 1, 2), gemm_order=(0, 1, 2, 3, 6, 7, 4, 5),
           sc_bufs=2, diag_assist=True, agg_evac_dve=False, wt_ki_split=False,
           warmup=8):
    """pe_pairs: sample-pairs whose expert-sum runs on TensorE (matmul
    accumulate into PSUM; with diag_assist the lhsT is diag(r) built by
    cheap [128,128] tensor_scalars and the rhs is the RAW weight tile, so
    assist pairs skip the full-width mults).  act_mult_es: experts whose
    scale-mult runs on ScalarE (DVE pairs only).  pool_add_lvls: add-tree
    levels (0..2) routed to Pool (HW-risky).  out_split: output DMAs per
    sample.  combine_order: pair emission order.  gemm_order: sample
    emission order for the GEMM.  agg_evac_dve: assist agg PSUM->SBUF evac
    on DVE instead of Act."""
    import concourse.tile as tile
    import concourse.mybir as mybir
    from concourse import bacc
    from concourse.tile import add_dep_helper

    f32 = mybir.dt.float32
    f16 = mybir.dt.float16
    mult = mybir.AluOpType.mult
    add = mybir.AluOpType.add

    dio = f16 if use_f16 else f32

    nc = bacc.Bacc("TRN2", target_bir_lowering=False, debug=False)
    x_d = nc.dram_tensor("x", [BPC, KI, 128, HW], dio, kind="ExternalInput")
    rw_d = nc.dram_tensor("rw", [1, BPC * E], f32, kind="ExternalInput")
    wt_d = nc.dram_tensor("wt", [128, E * WCOL], dio, kind="ExternalInput")
    id_d = nc.dram_tensor("ident", [128, 128], dio, kind="ExternalInput")
    out_d = nc.dram_tensor("out", [reps * BPC, MO, 128, HW], dio,
                           kind="ExternalOutput")

    n_pairs = BPC // 2
    pe_samples = {2 * p for p in pe_pairs} | {2 * p + 1 for p in pe_pairs}

    with tile.TileContext(nc) as tc:
        with ExitStack() as ctx:
            wt_pool = ctx.enter_context(tc.tile_pool(name="wt", bufs=1))
            rw_pool = ctx.enter_context(tc.tile_pool(name="rw", bufs=2))
            id_pool = ctx.enter_context(tc.tile_pool(name="id", bufs=2))
            sc_pool = ctx.enter_context(tc.tile_pool(name="sc", bufs=sc_bufs))
            agg_pool = ctx.enter_context(tc.tile_pool(name="agg", bufs=4))
            x_pool = ctx.enter_context(tc.tile_pool(name="xp", bufs=BPC))
            out_pool = ctx.enter_context(tc.tile_pool(name="op", bufs=3))
            aps_pool = ctx.enter_context(tc.tile_pool(name="aps", bufs=2,
                                                      space="PSUM"))
            ps_pool = ctx.enter_context(tc.tile_pool(name="ps", bufs=3,
                                                     space="PSUM"))

            prev_out_dmas, cur_out_dmas = [], []

            def _fence(inst):
                if serialize_reps:
                    for d in prev_out_dmas:
                        add_dep_helper(inst.ins, d.ins, reason="serialize reps")
                return inst

            for rep in range(reps):
                prev_out_dmas, cur_out_dmas = cur_out_dmas, []

                # ---- PE clock pre-warm: dummy matmuls on scratch data ----
                # The tensor engine ramps to full clock only after ~3us of
                # continuous execution; fill the DMA-bound head with junk
                # matmuls so real work starts at full speed.
                if warmup and rep == 0:
                    wsc = sc_pool.tile([128, 512], dio, tag="wsc",
                                       name=f"warm_{rep}")
                    nc.vector.memset(wsc[:], 0.0)
                    for wi in range(warmup):
                        wps = aps_pool.tile([128, C_OUT], f32, tag="aps",
                                            name=f"wps_{rep}_{wi}")
                        nc.tensor.matmul(wps[:], wsc[:, 0:128],
                                         wsc[:, 0:C_OUT],
                                         start=True, stop=True)

                # ---- per-rep loads ----
                rw_sb = rw_pool.tile([128, BPC * E], f32)
                _fence(nc.sync.dma_start(
                    rw_sb[:], rw_d[:].to_broadcast((128, BPC * E))))
                id_sb = id_pool.tile([128, 128], dio)
                _fence(nc.sync.dma_start(id_sb[:], id_d[:]))
                wt_sb = [wt_pool.tile([128, WCOL], dio, tag=f"wt{e}",
                                      name=f"wt{e}_{rep}")
                         for e in range(E)]
                if wt_ki_split:
                    # ki-major arrival: every expert's k0 slice lands first,
                    # so assist PSUM chains can stop (and evac) early
                    for ki in range(KI):
                        for e in range(E):
                            _fence(nc.sync.dma_start(
                                wt_sb[e][:, ki * C_OUT:(ki + 1) * C_OUT],
                                wt_d[:, e * WCOL + ki * C_OUT:
                                     e * WCOL + (ki + 1) * C_OUT]))
                else:
                    for e in range(E):
                        _fence(nc.sync.dma_start(
                            wt_sb[e][:], wt_d[:, e * WCOL:(e + 1) * WCOL]))
                x_sb = []
                for b in range(BPC):
                    xt = x_pool.tile([128, KI * HW], dio, tag="xt",
                                     name=f"x_{rep}_{b}")
                    _fence(nc.sync.dma_start(
                        xt[:], x_d[b].transpose([1, 0, 2])))
                    x_sb.append(xt)

                def sc_ap(b, e):
                    return rw_sb[:, b * E + e:b * E + e + 1]

                # ---- emission helpers ----
                aggs = {}
                dgs = {}  # (b, e) -> diag tile

                def _agg(pair):
                    if pair not in aggs:
                        aggs[pair] = agg_pool.tile(
                            [128, 2 * WCOL], dio, tag="agg",
                            name=f"agg_{rep}_{pair}")
                    return aggs[pair]

                def emit_dgs(pair):
                    # diag(r_be) tiles: cheap [128,128] scales of I on DVE
                    for half, b in ((0, 2 * pair), (1, 2 * pair + 1)):
                        for e in range(E):
                            dg = sc_pool.tile([128, 128], dio,
                                              tag=f"dg{half}{e}",
                                              name=f"dg_{rep}_{b}_{e}")
                            nc.vector.tensor_scalar(
                                dg[:], id_sb[:], sc_ap(b, e), None, mult)
                            dgs[(b, e)] = dg

                def emit_assist_mm(pair):
                    # psum += diag(r_be) @ W_e, evac to agg (Act)
                    agg = _agg(pair)
                    for half, b in ((0, 2 * pair), (1, 2 * pair + 1)):
                        for ki in range(KI):
                            aps = aps_pool.tile([128, C_OUT], f32, tag="aps",
                                                name=f"aps_{rep}_{b}_{ki}")
                            for e in range(E):
                                nc.tensor.matmul(
                                    aps[:], dgs[(b, e)][:],
                                    wt_sb[e][:, ki * C_OUT:(ki + 1) * C_OUT],
                                    start=(e == 0), stop=(e == E - 1))
                            dst = agg[:, half * WCOL + ki * C_OUT:
                                      half * WCOL + (ki + 1) * C_OUT]
                            if agg_evac_dve:
                                nc.vector.tensor_scalar(
                                    dst, aps[:], 1.0, None, mult)
                            else:
                                nc.scalar.copy(dst, aps[:])

                def emit_act_mults(pair, sct):
                    for e in act_mult_es:
                        for half, b in ((0, 2 * pair), (1, 2 * pair + 1)):
                            nc.scalar.mul(
                                sct[e][:, half * WCOL:(half + 1) * WCOL],
                                wt_sb[e][:], sc_ap(b, e))

                def emit_dve_mults(pair, sct):
                    for e in range(E):
                        if e in act_mult_es:
                            continue
                        for half, b in ((0, 2 * pair), (1, 2 * pair + 1)):
                            nc.vector.tensor_scalar(
                                sct[e][:, half * WCOL:(half + 1) * WCOL],
                                wt_sb[e][:], sc_ap(b, e), None, mult)

                def emit_adds(pair, sct):
                    agg = _agg(pair)

                    def _tt(lvl, dst, a, bb):
                        eng = (nc.gpsimd if lvl in pool_add_lvls
                               else nc.vector)
                        eng.tensor_tensor(dst, a, bb, add)
                    w2 = 2 * WCOL
                    _tt(0, sct[0][:, :w2], sct[0][:, :w2], sct[1][:, :w2])
                    _tt(0, sct[2][:, :w2], sct[2][:, :w2], sct[3][:, :w2])
                    _tt(0, sct[4][:, :w2], sct[4][:, :w2], sct[5][:, :w2])
                    _tt(0, sct[6][:, :w2], sct[6][:, :w2], sct[7][:, :w2])
                    _tt(1, sct[0][:, :w2], sct[0][:, :w2], sct[2][:, :w2])
                    _tt(1, sct[4][:, :w2], sct[4][:, :w2], sct[6][:, :w2])
                    _tt(2, agg[:, :w2], sct[0][:, :w2], sct[4][:, :w2])

                def alloc_sct(pair):
                    return [sc_pool.tile([128, 2 * WCOL], dio, tag=f"sc{e}",
                                         name=f"sc{e}_{rep}_{pair}")
                            for e in range(E)]

                def emit_gemm(b, dve_evac=False):
                    pair, half = b // 2, b % 2
                    agg = aggs[pair]
                    o_sb = out_pool.tile([128, MO * HW], dio, tag="osb",
                                         name=f"o_{rep}_{b}")
                    done_dmas = 0
                    for mo in range(MO):
                        pss = [ps_pool.tile([128, NCH], f32, tag=f"ps{n}",
                                            name=f"ps_{rep}_{b}_{mo}_{n}")
                               for n in range(NSPLIT)]
                        for ki in range(KI):
                            lhs = agg[:, half * WCOL + ki * C_OUT + mo * 128:
                                      half * WCOL + ki * C_OUT + (mo + 1) * 128]
                            for n in range(NSPLIT):
                                rhs = x_sb[b][:, ki * HW + n * NCH:
                                              ki * HW + (n + 1) * NCH]
                                nc.tensor.matmul(
                                    pss[n][:], lhs, rhs,
                                    start=(ki == 0), stop=(ki == KI - 1))
                        for n in range(NSPLIT):
                            dst = o_sb[:, mo * HW + n * NCH:
                                       mo * HW + (n + 1) * NCH]
                            if dve_evac and n == 1:
                                # tail: DVE is idle, parallelize the evac
                                nc.vector.tensor_scalar(
                                    dst, pss[n][:], 1.0, None, mult)
                            else:
                                nc.scalar.copy(dst, pss[n][:])
                        # flush output when a DMA-chunk worth is ready
                        if (out_split == 3 or
                                (out_split == 2 and mo == 0)):
                            cur_out_dmas.append(nc.sync.dma_start(
                                out_d[rep * BPC + b, mo:mo + 1]
                                .transpose([1, 0, 2]),
                                o_sb[:, mo * HW:(mo + 1) * HW]))
                            done_dmas = mo + 1
                    if done_dmas < MO:
                        lo = done_dmas * HW
                        cur_out_dmas.append(nc.sync.dma_start(
                            out_d[rep * BPC + b, done_dmas:MO]
                            .transpose([1, 0, 2]),
                            o_sb[:, lo:MO * HW]))

                # ---- program: slack-first emission ----
                # DVE queue: dgs(all assist) | p1 mults+adds | p2 mults+adds
                # PE queue: assist p0 | G0 G1 | assist p3 | G2 G3 G6 G7 G4 G5
                # Act queue: act mults p1 p2 | agg evacs p0 | out evacs ...
                assist = list(pe_pairs)
                dve_pairs = [p for p in combine_order if p not in assist]
                for p in assist:
                    emit_dgs(p)
                scts = {p: alloc_sct(p) for p in dve_pairs}
                for p in dve_pairs:
                    emit_act_mults(p, scts[p])
                if dve_pairs:
                    p = dve_pairs[0]
                    emit_dve_mults(p, scts[p])
                    emit_adds(p, scts[p])
                if assist:
                    emit_assist_mm(assist[0])
                for i, b in enumerate(gemm_order):
                    emit_gemm(b, dve_evac=(i >= len(gemm_order) - 2))
                    if i == 1:
                        for p in assist[1:]:
                            emit_assist_mm(p)
                        for p in dve_pairs[1:]:
                            emit_dve_mults(p, scts[p])
                            emit_adds(p, scts[p])
    nc.compile()
    return nc


def _host_inputs(x, routing_weights, weight):
    """Full fp32 inputs -> list of per-core input maps (wire dtype)."""
    np_io = np.float16 if USE_F16 else np.float32
    x_r = np.ascontiguousarray(
        x.reshape(B, C_IN, HW).astype(np_io).reshape(B, KI, 128, HW))
    # wt[p, e*1152 + ki*384 + o] = weight[e, o, ki*128 + p]
    wt = np.ascontiguousarray(
        weight.reshape(E, C_OUT, KI, 128).transpose(3, 0, 2, 1)
        .reshape(128, E * WCOL).astype(np_io))
    ident = np.ascontiguousarray(np.eye(128, dtype=np_io))
    in_maps = []
    for c in range(N_CORES):
        sl = slice(c * BPC, (c + 1) * BPC)
        in_maps.append({
            "x": x_r[sl],
            "rw": np.ascontiguousarray(
                routing_weights[sl].reshape(1, BPC * E).astype(np.float32)),
            "wt": wt,
            "ident": ident,
        })
    return in_maps


def kernel(x: np.ndarray, routing_weights: np.ndarray, weight: np.ndarray,
           _trace: bool = False):
    from concourse.bass_utils import run_bass_kernel_spmd

    x = np.asarray(x, dtype=np.float32)
    routing_weights = np.ascontiguousarray(
        np.asarray(routing_weights, dtype=np.float32))
    weight = np.asarray(weight, dtype=np.float32)

    if "nc" not in _cache:
        _cache["nc"] = _build()
    nc = _cache["nc"]

    in_maps = _host_inputs(x, routing_weights, weight)
    res = run_bass_kernel_spmd(nc, in_maps, core_ids=list(range(N_CORES)),
                               trace=_trace)
    out = np.concatenate(
        [res.results[c]["out"].reshape(BPC, C_OUT, HW)
         for c in range(N_CORES)], axis=0)
    if _trace:
        _cache["last_result"] = res
    return out.reshape(B, C_OUT, H, W).astype(np.float32)


def _bench_inputs(rng, reps=1):
    """Random per-core input map matching the dram tensor shapes."""
    np_io = np.float16 if USE_F16 else np.float32
    return {
        "x": rng.standard_normal((BPC, KI, 128, HW),
                                 dtype=np.float32).astype(np_io),
        "rw": rng.random((1, BPC * E), dtype=np.float32),
        "wt": rng.standard_normal((128, E * WCOL),
                                  dtype=np.float32).astype(np_io),
        "ident": np.eye(128, dtype=np_io),
    }


if __name__ == "__main__":
    rng = np.random.default_rng(0)
    x = rng.standard_normal((B, C_IN, H, W), dtype=np.float32)
    rw = rng.random((B, E), dtype=np.float32)
    w = rng.standard_normal((E, C_OUT, C_IN), dtype=np.float32)
    got = kernel(x, rw, w)
    agg = np.einsum('be,eoi->boi', rw, w)
    want = np.einsum('boi,bihw->bohw', agg, x.reshape(B, C_IN, H, W))
    err = np.abs(got - want).max() / np.abs(want).max()
    print("rel err:", err)
